# revision 18
# baseline (speedup 1.0000x reference)
"""Trainium2 Bass kernel for the ConditionalAffineCouplingLayer problem.

8 NeuronCores, pure data parallel over the batch:
  - Host: fold positional encoding into x, split even/odd features,
    transpose to feature-major layout for contiguous DMA.
  - Pass 1 (fp32r): fused forward per shard where each 2048-row group is
    normalized by its own group-local BN stats (self-consistent batchnorm);
    records exact per-layer bn_stats of the pass-1 trajectory.
  - One 16KB AllReduce combines per-layer (sum, sumsq) across the 8 cores.
  - Pass 2 (fp32): exact forward using the synced global stats.
  The map "stats -> stats of trajectory normalized by them" contracts by
  ~100x per iteration, so pass-2 matches full-batch BN semantics to ~1e-4.
"""

import numpy as np
import sys

sys.path.insert(0, "/opt/trn_rl_repo")

import concourse.bass as bass
import concourse.bacc as bacc
import concourse.tile as tile
from concourse import mybir
from concourse.bass_utils import run_bass_kernel_spmd

F32 = mybir.dt.float32
F32R = mybir.dt.float32r
BF16 = mybir.dt.bfloat16
AF = mybir.ActivationFunctionType
ALU = mybir.AluOpType
AX = mybir.AxisListType

B, D, C, H = 131072, 96, 32, 128
IN, OUT = D // 2 + C, D // 2  # 80, 48
SF = 2.0
EPS = 1e-5
ALPHA = 0.01  # leaky_relu slope
NCORES = 8
SHARD = B // NCORES  # 16384
G = 2048  # rows per group
NMM = G // 512  # matmuls per group
NL = 16  # BN layers total (4 nets x 4)

# engine pattern for BN applies: True -> ACT Lrelu (1 op), False -> DVE (2 ops)
APPLY_PATTERN = [True, True, True, False, True, True, False]

_CACHE = {}


def _lrelu(v):
    return np.where(v > 0, v, ALPHA * v)


def _build(nc, act_lrelu=True, shard=SHARD, use_f32r=True, ncores=NCORES,
           niter=1):
    ng = shard // G

    FR = F32R if use_f32r else F32
    T = {}
    T["xe_d"] = nc.dram_tensor("xe", [OUT, shard], FR, kind="ExternalInput")
    T["xo_d"] = nc.dram_tensor("xo", [OUT, shard], F32, kind="ExternalInput")
    T["cd_d"] = nc.dram_tensor("cond", [C, shard], FR, kind="ExternalInput")
    T["w2l1_d"] = nc.dram_tensor("w2l1", [4, IN, H], FR, kind="ExternalInput")
    T["w2mid_d"] = nc.dram_tensor("w2mid", [4, 3, H, H], FR, kind="ExternalInput")
    T["w2f_d"] = nc.dram_tensor("w2f", [4, H, OUT], FR, kind="ExternalInput")
    T["b_d"] = nc.dram_tensor("bpack", [H, NL], F32, kind="ExternalInput")
    T["g_d"] = nc.dram_tensor("gpack", [H, NL], F32, kind="ExternalInput")
    T["be_d"] = nc.dram_tensor("bepack", [H, NL], F32, kind="ExternalInput")
    T["bf_d"] = nc.dram_tensor("bfpack", [H, 4], F32, kind="ExternalInput")

    T["ye_d"] = nc.dram_tensor("ye", [OUT, shard], F32, kind="ExternalOutput")
    T["yo_d"] = nc.dram_tensor("yo", [OUT, shard], F32, kind="ExternalOutput")
    T["ld_d"] = nc.dram_tensor("ld", [OUT, 1], F32, kind="ExternalOutput")
    T["gs_d"] = nc.dram_tensor("gsums", [H, 2 * NL], F32, kind="ExternalOutput")

    with tile.TileContext(nc) as tc:
        for _ in range(niter):
            _emit(tc, nc, T, act_lrelu, ng, use_f32r, shard, ncores)
    return nc


def _emit(tc, nc, T, act_lrelu, ng, use_f32r, shard, ncores):
    from contextlib import ExitStack

    FR = F32R if use_f32r else F32

    ctx = ExitStack()
    wpool = ctx.enter_context(tc.tile_pool(name="w", bufs=1))
    cpool = ctx.enter_context(tc.tile_pool(name="consts", bufs=1))
    spool = ctx.enter_context(tc.tile_pool(name="stats", bufs=1))
    apool = ctx.enter_context(tc.tile_pool(name="acts", bufs=2))
    inpool = ctx.enter_context(tc.tile_pool(name="ins", bufs=2))
    psum = ctx.enter_context(tc.tile_pool(name="psum", bufs=2, space="PSUM"))
    dram = ctx.enter_context(tc.tile_pool(name="dram", bufs=1, space="DRAM"))

    # ---- weights ----
    w2 = {}
    for n in range(4):
        t = wpool.tile([IN, H], FR, tag=f"w2l1_{n}")
        nc.sync.dma_start(t[:], T["w2l1_d"][n])
        w2[(n, 0)] = t
        for k in range(3):
            t = wpool.tile([H, H], FR, tag=f"w2m_{n}_{k}")
            nc.sync.dma_start(t[:], T["w2mid_d"][n, k])
            w2[(n, k + 1)] = t
        t = wpool.tile([H, OUT], FR, tag=f"w2f_{n}")
        nc.sync.dma_start(t[:], T["w2f_d"][n])
        w2[(n, 4)] = t
    w1 = w2

    # ---- packed constants ----
    def load_const(key, shape):
        t = cpool.tile(shape, F32, tag=key)
        nc.sync.dma_start(t[:], T[key + "_d"][:])
        return t

    bp = load_const("b", [H, NL])
    gp = load_const("g", [H, NL])
    bep = load_const("be", [H, NL])
    bfp = load_const("bf", [H, 4])

    # persistent stats stash: per layer [128, ng*24] bn_stats outputs
    stash = [spool.tile([H, ng * 24], F32, tag=f"st{l}", name=f"st{l}")
             for l in range(NL)]
    a2p = spool.tile([H, NL], F32, tag="a2p")
    bias2p = spool.tile([H, NL], F32, tag="bias2p")
    ldslots = spool.tile([OUT, 2 * ng], F32, tag="ldslots")

    fpool = ctx.enter_context(tc.tile_pool(name="fin", bufs=3))
    epsb0 = spool.tile([H, 1], F32, tag="epsb0")
    nc.gpsimd.memset(epsb0[:], EPS)

    apply_ctr = [0]

    def bn_apply(z_ap, out_ap, scale_ap, bias_ap):
        """out = lrelu(z*scale + bias); z in PSUM."""
        use_act = act_lrelu and APPLY_PATTERN[apply_ctr[0] % len(APPLY_PATTERN)]
        apply_ctr[0] += 1
        if use_act:
            nc.scalar.activation(out_ap, z_ap, AF.Lrelu,
                                 bias=bias_ap, scale=scale_ap, alpha=ALPHA)
        else:
            nc.vector.tensor_scalar(out_ap, z_ap, scale_ap, bias_ap,
                                    ALU.mult, ALU.add)
            nc.vector.scalar_tensor_tensor(out_ap, out_ap, ALPHA, out_ap,
                                           ALU.mult, ALU.max)

    def mm(out_ap, w_tile, rhs_tile_ap, kdim, p):
        for m in range(NMM):
            nc.tensor.matmul(out_ap[:, bass.ts(m, 512)], w_tile[:],
                             rhs_tile_ap[:kdim, bass.ts(m, 512)])

    def record(l, g, z):
        for c in range(NMM):
            i = (g * NMM + c) * 6
            nc.vector.bn_stats(stash[l][:, i:i + 6], z[:, bass.ts(c, 512)])

    def local_stats(l, g, gam_ap, be_ap):
        """Group-local BN params from the just-recorded bn_stats partials.
        Returns (scale_ap, bias_ap) applying to z (bias-less h)."""
        loc = fpool.tile([H, 2], F32, tag="loc")
        nc.vector.bn_aggr(loc[:], stash[l][:, g * 24:(g + 1) * 24])
        sd = fpool.tile([H, 1], F32, tag="sd_l")
        nc.scalar.activation(sd[:], loc[:, 1:2], AF.Sqrt, bias=epsb0[:])
        rs = fpool.tile([H, 1], F32, tag="rs_l")
        nc.vector.reciprocal(rs[:], sd[:])
        sc = fpool.tile([H, 1], F32, tag="sc_l")
        nc.vector.tensor_tensor(sc[:], rs[:], gam_ap, ALU.mult)
        bi = fpool.tile([H, 1], F32, tag="bi_l")
        nc.vector.tensor_scalar(bi[:], loc[:, 0:1], sc[:], -1.0,
                                ALU.mult, ALU.mult)
        nc.vector.tensor_tensor(bi[:], bi[:], be_ap, ALU.add)
        return sc, bi

    def net_forward(p, n, in_ap, g, rec, dt, skip_final=False):
        """in_ap: [IN partitions, G]. Returns final-linear PSUM tile (or None)."""
        wmap = w1 if p == 1 else w2
        cur_ap = in_ap
        kdim = IN
        for k in range(4):
            l = n * 4 + k
            z = psum.tile([H, G], F32, tag="z")
            mm(z[:], wmap[(n, k)], cur_ap, kdim, p)
            if rec:
                record(l, g, z)
            if k == 3 and skip_final:
                return None
            a = apool.tile([H, G], FR, tag=f"a{n % 2}")
            if p == 1:
                sc, bi = local_stats(l, g, gp[:, l:l + 1], bep[:, l:l + 1])
                bn_apply(z[:], a[:], sc[:], bi[:])
            else:
                bn_apply(z[:], a[:], a2p[:, l:l + 1], bias2p[:, l:l + 1])
            cur_ap = a[:]
            kdim = H
        o = psum.tile([H, G], F32, tag="z")
        mm(o[:OUT, :], wmap[(n, 4)], cur_ap, H, p)
        return o

    def coupling_half(p, half, in_ap, g, rec, dt, x_ap, out_ap, ld_idx):
        """nets (2h, 2h+1): out = x * exp(s) + t  (written to out_ap)."""
        n_s, n_t = 2 * half, 2 * half + 1
        o_s = net_forward(p, n_s, in_ap, g, rec, dt)
        spre = apool.tile([OUT, G], F32, tag="tmp48", bufs=1)
        kwargs = {}
        if p == 2:
            kwargs["accum_out"] = ldslots[:, ld_idx:ld_idx + 1]
        nc.scalar.activation(spre[:], o_s[:OUT, :], AF.Arctan,
                             bias=bfp[:OUT, n_s:n_s + 1], scale=1.0 / SF,
                             **kwargs)
        E = apool.tile([OUT, G], dt, tag="E")
        nc.scalar.activation(E[:], spre[:], AF.Exp, scale=2.0 * SF / np.pi)
        o_t = net_forward(p, n_t, in_ap, g, rec, dt)
        tt = apool.tile([OUT, G], dt, tag="tt")
        nc.scalar.activation(tt[:], o_t[:OUT, :], AF.Identity,
                             bias=bfp[:OUT, n_t:n_t + 1])
        tmp = apool.tile([OUT, G], dt, tag="tmp48", bufs=1)
        if x_ap.dtype == F32R:
            x_ap = x_ap.bitcast(F32)
        nc.vector.tensor_tensor(tmp[:], x_ap, E[:], ALU.mult)
        nc.vector.tensor_tensor(out_ap, tmp[:], tt[:], ALU.add)

    # ================= PASS 1 (f32, approx stats, record bn_stats) =========
    for g in range(ng):
        gsl = bass.ts(g, G)
        in1 = inpool.tile([IN, G], FR, tag="in1")
        nc.sync.dma_start(in1[0:OUT, :], T["xe_d"][:, gsl])
        nc.sync.dma_start(in1[OUT:IN, :], T["cd_d"][:, gsl])
        xo2 = inpool.tile([OUT, G], F32, tag="xo2")
        nc.sync.dma_start(xo2[:], T["xo_d"][:, gsl])
        in2 = inpool.tile([IN, G], FR, tag="in2")
        nc.sync.dma_start(in2[OUT:IN, :], T["cd_d"][:, gsl])

        coupling_half(1, 0, in1[0:IN, :], g, True, F32,
                      xo2[:], in2[0:OUT, :], 0)
        # half 2: stats only
        net_forward(1, 2, in2[0:IN, :], g, True, F32, skip_final=True)
        net_forward(1, 3, in2[0:IN, :], g, True, F32, skip_final=True)

    # ================= aggregate + AllReduce ===============================
    statsP = spool.tile([H, NL, 2], F32, tag="statsP")
    for l in range(NL):
        nc.vector.bn_aggr(statsP[:, l, :], stash[l][:])
    muz = statsP[:, :, 0]
    varz = statsP[:, :, 1]
    muh = spool.tile([H, NL], F32, tag="muh")
    nc.vector.tensor_tensor(muh[:], muz, bp[:], ALU.add)
    send = spool.tile([H, 2 * NL], F32, tag="send")
    nc.vector.tensor_scalar(send[:, 0:NL], muh[:], float(shard), None, ALU.mult)
    msq = spool.tile([H, NL], F32, tag="msq")
    nc.vector.tensor_tensor(msq[:], muh[:], muh[:], ALU.mult)
    nc.vector.tensor_tensor(msq[:], msq[:], varz, ALU.add)
    nc.vector.tensor_scalar(send[:, NL:2 * NL], msq[:], float(shard), None,
                            ALU.mult)

    in_bounce = dram.tile([H, 2 * NL], F32)
    out_bounce = dram.tile([H, 2 * NL], F32)
    nc.sync.dma_start(in_bounce[:], send[:])
    import os as _os
    if _os.environ.get("SKIP_CC", "") == "1":
        nc.sync.dma_start(out_bounce[:], in_bounce[:])
    else:
        nc.gpsimd.collective_compute(
            "AllReduce", ALU.add,
            replica_groups=[list(range(ncores))],
            ins=[in_bounce.opt()],
            outs=[out_bounce.opt()],
        )
    gsum = spool.tile([H, 2 * NL], F32, tag="gsum")
    nc.sync.dma_start(gsum[:], out_bounce[:])
    nc.sync.dma_start(T["gs_d"][:], gsum[:])

    inv_n = 1.0 / float(shard * ncores)
    gmu = spool.tile([H, NL], F32, tag="gmu")
    nc.vector.tensor_scalar(gmu[:], gsum[:, 0:NL], inv_n, None, ALU.mult)
    gvar = spool.tile([H, NL], F32, tag="gvar")
    nc.vector.tensor_scalar(gvar[:], gsum[:, NL:2 * NL], inv_n, None, ALU.mult)
    gmsq = spool.tile([H, NL], F32, tag="gmsq")
    nc.vector.tensor_tensor(gmsq[:], gmu[:], gmu[:], ALU.mult)
    nc.vector.tensor_tensor(gvar[:], gvar[:], gmsq[:], ALU.subtract)
    epsb = spool.tile([H, 1], F32, tag="epsb")
    nc.gpsimd.memset(epsb[:], EPS)
    sd = spool.tile([H, NL], F32, tag="sd")
    nc.scalar.activation(sd[:], gvar[:], AF.Sqrt, bias=epsb[:])
    rs = spool.tile([H, NL], F32, tag="rs")
    nc.vector.reciprocal(rs[:], sd[:])
    nc.vector.tensor_tensor(a2p[:], gp[:], rs[:], ALU.mult)
    tmpv = spool.tile([H, NL], F32, tag="tmpv")
    nc.vector.tensor_tensor(tmpv[:], bp[:], gmu[:], ALU.subtract)
    nc.vector.tensor_tensor(tmpv[:], tmpv[:], a2p[:], ALU.mult)
    nc.vector.tensor_tensor(bias2p[:], tmpv[:], bep[:], ALU.add)

    # ================= PASS 2 (fp32, exact) ================================
    for g in range(ng):
        gsl = bass.ts(g, G)
        # in1: 0-47 x11, 48-79 cond; xo separate tile
        in1 = inpool.tile([IN, G], FR, tag="in1")
        nc.sync.dma_start(in1[0:OUT, :], T["xe_d"][:, gsl])
        nc.sync.dma_start(in1[OUT:IN, :], T["cd_d"][:, gsl])
        xo2 = inpool.tile([OUT, G], F32, tag="xo2")
        nc.sync.dma_start(xo2[:], T["xo_d"][:, gsl])
        in2 = inpool.tile([IN, G], FR, tag="in2")
        nc.sync.dma_start(in2[OUT:IN, :], T["cd_d"][:, gsl])

        coupling_half(2, 0, in1[0:IN, :], g, False, F32, xo2[:],
                      in2[0:OUT, :], 2 * g)
        nc.sync.dma_start(T["yo_d"][:, gsl], in2[0:OUT, :].bitcast(F32))
        ye_t = apool.tile([OUT, G], F32, tag="ye_t", bufs=1)
        coupling_half(2, 1, in2[0:IN, :], g, False, F32, in1[0:OUT, :],
                      ye_t[:], 2 * g + 1)
        nc.sync.dma_start(T["ye_d"][:, gsl], ye_t[:])

    ldv = spool.tile([OUT, 1], F32, tag="ldv")
    nc.vector.reduce_sum(ldv[:], ldslots[:], axis=AX.X)
    nc.sync.dma_start(T["ld_d"][:], ldv[:])

    ctx.close()


def _prep_inputs(inputs, shard=SHARD, ncores=NCORES):
    x = np.asarray(inputs["x"], np.float32)
    cond = np.asarray(inputs["condition"], np.float32)
    P = {k: np.asarray(v, np.float32) for k, v in inputs.items()}

    pe = np.sin(np.arange(D, dtype=np.float32))
    xp = x + pe
    xe = np.ascontiguousarray(xp[:, 0::2].T)
    xo = np.ascontiguousarray(xp[:, 1::2].T)
    cd = np.ascontiguousarray(cond.T)

    bp = np.zeros((H, NL), np.float32)
    gp = np.zeros((H, NL), np.float32)
    bep = np.zeros((H, NL), np.float32)
    for n in range(4):
        for k in range(4):
            l = n * 4 + k
            g_ = P["g0"][n] if k == 0 else P["gs"][n][k - 1]
            be_ = P["be0"][n] if k == 0 else P["bes"][n][k - 1]
            b_ = P["b0"][n] if k == 0 else P["bs"][n][k - 1]
            bp[:, l] = b_
            gp[:, l] = g_
            bep[:, l] = be_
    bfp = np.zeros((H, 4), np.float32)
    for n in range(4):
        v = P["bf"][n]
        bfp[:OUT, n] = v / SF if n in (0, 2) else v

    common = dict(
        w2l1=P["W0"], w2mid=P["Ws"], w2f=P["Wf"],
        bpack=bp, gpack=gp, bepack=bep, bfpack=bfp,
    )
    in_maps = []
    for i in range(ncores):
        sl = slice(i * shard, (i + 1) * shard)
        m = dict(common)
        m["xe"] = np.ascontiguousarray(xe[:, sl])
        m["xo"] = np.ascontiguousarray(xo[:, sl])
        m["cond"] = np.ascontiguousarray(cd[:, sl])
        in_maps.append(m)
    return in_maps


def _get_nc(act_lrelu=True, use_f32r=True):
    key = (act_lrelu, use_f32r)
    if key not in _CACHE:
        nc = bacc.Bacc("TRN2", target_bir_lowering=False, debug=False,
                       num_devices=NCORES)
        _build(nc, act_lrelu=act_lrelu, use_f32r=use_f32r)
        nc.compile()
        _CACHE[key] = nc
    return _CACHE[key]


def kernel(**inputs):
    in_maps = _prep_inputs(inputs)
    nc = _get_nc()
    res = run_bass_kernel_spmd(nc, in_maps, list(range(NCORES)))
    return _gather(res.results, B)


def _gather(results, nrows):
    ye = np.concatenate([np.asarray(r["ye"]) for r in results], axis=1)
    yo = np.concatenate([np.asarray(r["yo"]) for r in results], axis=1)
    y = np.empty((nrows, D), np.float32)
    y[:, 0::2] = ye.T
    y[:, 1::2] = yo.T
    ld = sum(float(np.asarray(r["ld"]).sum()) for r in results)
    log_det = np.float32(ld * (2.0 * SF / np.pi) / nrows)
    return y, log_det
